# revision 74
# baseline (speedup 1.0000x reference)
"""AR(16) sampling kernel for 8 TRN2 NeuronCores.

Math: the reference scan y_t = sum_j a_j y_{t-j} + eps_t is, to f32
accuracy, a 256-tap causal FIR of the noise (the AR poly's roots lie
inside |z| <= 0.91 so the impulse response h is < 1e-9 by lag 128,
1e-18 by 256) plus a decaying response to the initial state:

    y_t = sum_d h[d] * std * noise2[t-d]  +  sum_i G[i, t] * iv[b, i]

with noise2 = noise zero-padded by n=16 rows at the front.

Device formulation (time-major, H-stationary): output time-chunk
(128 steps x 512 batch) = one full bf16 matmul (D0, lags 0..127 within
the chunk) plus a HALF-WIDTH matmul for the cross-chunk D1 term, which
only matters for outputs t<64 (for t>=64 its variance share is <1e-5
and is dropped). Chunk pairs share one PE pass for their two half D1
matmuls via column tiling: T0 (psum partitions 0-63) computes the even
chunk's D1 term while T1 (partitions 64-127) concurrently computes the
odd chunk's, whose whole psum layout is rotated by 64 partitions (D0r
= column-rotated D0; the host un-rotates odd chunks on decode). A
T0/T1 pair takes ~216 ns - the time of ONE full matmul (hw-probed) -
so each chunk costs 1.5 instead of 2 full PE passes.

The initial-state response is folded into the FIR: noise chunk 0's
zero-padding rows 0..15 carry the initial values and chunks 0/1 use
G-spliced stationaries (D0p/D1ph) - no separate G matmuls.

Schedule (everything tuned against hw traces):
- chunks in groups of 4 (plus two 2-chunk tail groups that drain with
  one parallel single-cast per engine), each group emitting three
  same-weight runs (D0 x2, D0r x2, D1-half-pairs x2) so the stationary
  changes only 3 times per 6 PE passes (per-pair interleaving exposes
  the weight reloads - measured +2.7 us);
- even groups accumulate in PSUM banks 0-3, odd in 4-7: adjacent
  groups share no banks, so a group's casts never stall the next
  group's matmuls;
- PSUM is evacuated with 1024-col bank-PAIR casts, ACT + DVE one each
  per group (ACT's PSUM reads are free; concurrent DVE reads slow the
  PE writeback 216 -> 375 ns, so DVE gets one window per group);
- one output stripe per group, never reused: a single slow store
  receipt cannot head-of-line-block the strict-FIFO cast queues;
- ~62 tiny warmup matmuls emitted RAW (before the TileContext, so
  they carry no semaphores and start right at the ~7 us framework
  preamble boundary) ride out the HAM's 1.2 GHz cold clock (it
  un-throttles after ~3.4 us of sustained PE activity, and any
  >0.5 us idle gap resets the ramp) and hand off to the first real
  matmul exactly when the first loads' DMA receipts land;
- loads+stores share the sync HWDGE ring (loads first; ring is FIFO),
  consts ride the scalar ring, and the cast engines issue no DMAs, so
  early casts never queue behind load issues.

Output is int8 everywhere: the quantization scale s = 18/127 is folded
into the matmul weights (D/s), PSUM already holds y/s, and the
PSUM->SBUF copy casts straight to int8 (hw-verified round-to-nearest-
even with saturation on both DVE and ACT; clipping |y|>18 trades rare
saturation error against a finer ulp - total rel err ~1.69e-2 vs the
2e-2 gate). Host decodes y = q * s. Traffic per core: 4.2 MB fp8 noise
in + 4.2 MB int8 out.

Sharding: pure data parallelism, batch split 8 ways (512 rows/core).
"""

import os
import sys

import numpy as np

sys.path.insert(0, "/opt/trn_rl_repo")

N_CORES = 8
B_FULL = 4096
N_AR = 16
STEPS = 8192
B_SHARD = B_FULL // N_CORES  # 512
P = 128
NCH = STEPS // P             # 64 time chunks per core
GRP = 4                      # chunks per group (2 groups in flight across the 8 psum banks)
SMAX = 18.0                  # int8 clip point; |y|>18 saturates
SCALE = SMAX / 127.0

LAST_RESULTS = None  # BassKernelResults of the most recent run (for test.py)


def _build_nc(Bs: int, nch: int):
    """Per-core Bass graph. Bs = batch shard, nch = time chunks."""
    import concourse.mybir as mybir
    from concourse import bacc
    from concourse.tile import TileContext

    f32 = mybir.dt.float32
    bf16 = mybir.dt.bfloat16
    fp8 = mybir.dt.float8e3
    i8 = mybir.dt.int8

    ngrp = nch // GRP
    assert ngrp * GRP == nch
    sizes = [2, 2, 4]
    while sum(sizes) < nch:
        sizes.append(min(GRP, nch - sum(sizes)))
    assert sum(sizes) == nch, sizes

    # const buffer [D0 | D1 | D0p | D1p]: the initial-state response is
    # folded into the FIR - noise chunk 0 rows 0..15 (zero padding in
    # the plain formulation) carry the initial values, and D0p/D1p are
    # D0/D1 with rows 0..15 replaced by the G-response blocks, used only
    # for chunks 0/1. No separate G matmuls needed.
    CW = 512
    nc = bacc.Bacc()
    npk_d = nc.declare_dram_parameter("npk", [P, nch * Bs], fp8, isOutput=False)
    cmb_d = nc.declare_dram_parameter("cmb", [P, CW], bf16, isOutput=False)
    out_d = nc.declare_dram_parameter("out", [P, nch * Bs], i8, isOutput=True)

    # PE warm-up BEFORE the TileContext: these raw matmuls carry no
    # semaphores, so they execute right after the tensor engine's
    # framework preamble (~5.4 us) - inside the Tile prologue window
    # that the PE otherwise spends idle - and the HAM reaches full
    # clock (~3.4 us of sustained activity) before the first real
    # matmul's data lands. They read uninitialized SBUF (worst case
    # NaN, which the PE propagates harmlessly into a psum bank that is
    # freed below and overwritten by the first start=True matmul).
    warm_raw = nc.alloc_sbuf_tensor("warmraw", [P, 64], bf16)
    with nc.psum_tensor("wpsraw", [64, 512], f32) as wraw:
        for i in range(62):
            nc.tensor.matmul(
                wraw[0:64, 0:64], lhsT=warm_raw[:, 0:64],
                rhs=warm_raw[:, 0:64], start=True, stop=True,
            )

    with TileContext(nc) as tc:
        with (
            tc.tile_pool(name="const", bufs=1) as cpool,
            # one slot per load group: every load pre-queues at kernel
            # start (the whole noise shard stays resident in SBUF)
            tc.tile_pool(name="noise", bufs=len(sizes)) as npool,
            tc.tile_pool(name="ostripe", bufs=nch // GRP + 1) as opool,
            tc.tile_pool(name="psum", bufs=4, space="PSUM") as ppool,
        ):
            # consts lead the scalar ring (first matmul needs D0p)
            cmb_t = cpool.tile([P, CW], bf16)
            nc.scalar.dma_start(out=cmb_t, in_=cmb_d[:, :])

            # noise loads: ramped sizes so the first chunks land right
            # after the engine preamble (~1.5 us after issue) and the PE
            # never starves; all pre-queued, alternating rings
            chunk_loc = {}
            c0 = 0
            for g, sz in enumerate(sizes):
                t = npool.tile(
                    [P, sz * Bs], fp8, tag="noise", name=f"nz{g}"
                )
                # loads on the sync ring, ahead of the stores (ring is
                # FIFO; stores only begin once loads have drained, which
                # is fine - the first store isn't ready before ~15 us).
                # Keeping loads off the scalar engine matters: ACT must
                # do nothing but casts, or the early groups' casts queue
                # behind 6.8 us of load issues and stall the psum banks.
                nc.sync.dma_start(
                    out=t, in_=npk_d[:, c0 * Bs : (c0 + sz) * Bs]
                )
                for r in range(sz):
                    chunk_loc[c0 + r] = (t, r)
                c0 += sz

            def view1(c):
                t, r = chunk_loc[c]
                return t[:, r * Bs : (r + 1) * Bs]

            # PE warm-up: HAM clocks the PE at 1.2 GHz until ~3.4 us of
            # sustained activity; run small matmuls round-robin over all
            # 8 psum banks (no WAW serialization) while the first noise
            # load lands, so the real stream starts at 2.4 GHz.
            # psum as 4 bank-PAIR tiles [128, 1024] (2 adjacent banks
            # each): matmuls write one-bank halves, the PSUM->SBUF cast
            # reads the whole pair in ONE 1024-col instruction - half
            # the cast instructions, half the windows in which a DVE
            # psum read slows the PE writeback (216 -> 375 ns/matmul)
            pspair = [
                ppool.tile([P, 2 * Bs], f32, tag="ps", name=f"psp{i}")
                for i in range(4)
            ]
            # one stripe per group, never reused: a cast can only ever
            # wait on its own stop-matmuls, so one slow store receipt
            # can't head-of-line-block the strict-FIFO cast queues
            stripes = [
                opool.tile([P, GRP * Bs], i8, tag="s8", name=f"st{i}")
                for i in range(ngrp + 1)
            ]

            # [D0 | D0r | D1h | D0p | D1ph]: D0r is D0 with columns
            # rotated by 64 (odd chunks' psum layout is rotated so both
            # halves of a chunk pair's D1 terms land on opposite PSUM
            # partition halves); D1h/D1ph are the t<64 half of D1/D1p
            # (the t>=64 D1 contribution is < 1e-5 of output variance
            # and is dropped)
            D0 = cmb_t[:, 0:P]
            D0r = cmb_t[:, P : 2 * P]
            D1h = cmb_t[:, 2 * P : 2 * P + 64]
            D0p = cmb_t[:, 2 * P + 64 : 3 * P + 64]
            D1ph = cmb_t[:, 3 * P + 64 : 4 * P]

            # 15 groups of 4 chunks + two 2-chunk tail groups: each tail
            # group drains with ONE single cast per engine in parallel,
            # so the post-stream drain is ~1 us shorter than a 4-chunk
            # final group (2 serialized casts per engine)
            groups = [(i * GRP, GRP) for i in range(15)] + [(60, 2), (62, 2)]
            for g, (c0, sz) in enumerate(groups):
                # even groups use pairs 0-1 (banks 0-3), odd groups 2-3:
                # adjacent groups touch disjoint bank sets, so group g+1's
                # matmuls never wait on group g's casts. The tail groups
                # take pairs 2 and 3, last used ~3.5 us earlier by g13.
                if sz == GRP:
                    pr = pspair[(g % 2) * 2 : (g % 2) * 2 + 2]
                else:
                    pr = [pspair[2 if g == 15 else 3]]
                ps = [pr[r // 2][:, (r % 2) * Bs : (r % 2 + 1) * Bs]
                      for r in range(sz)]
                # palindrome: even groups D0-run then D1-run, odd groups
                # D1-run then D0-run -> stationary changes once per 16
                # matmuls (the boundary LDW is identical & pre-pulled).
                # run A: full D0 matmuls on even chunks (standard psum
                # layout; chunk 0 uses the G-spliced D0p and closes
                # immediately); run B: full rotated-D0 matmuls on odd
                # chunks; run C: the D1 terms as column-TILED half
                # matmuls - T0 (psum partitions 0-63, even chunk) and
                # T1 (partitions 64-127, rotated odd chunk) stream
                # different noise chunks CONCURRENTLY (hw-probed: a
                # T0/T1 pair takes ~216 ns, the time of one full
                # matmul). Run-based order keeps weight switches to 3
                # per group - per-pair interleaving exposes the reloads.
                for r in range(0, sz, 2):
                    c = c0 + r
                    nc.tensor.matmul(
                        ps[r], lhsT=(D0p if c == 0 else D0),
                        rhs=view1(c), start=True, stop=(c == 0),
                    )
                for r in range(1, sz, 2):
                    nc.tensor.matmul(
                        ps[r], lhsT=D0r, rhs=view1(c0 + r),
                        start=True, stop=False,
                    )
                for r in range(0, sz, 2):
                    ca = c0 + r
                    if ca > 0:
                        nc.tensor.matmul(
                            ps[r][0:64, :], lhsT=D1h,
                            rhs=view1(ca - 1), start=False, stop=True,
                        )
                    nc.tensor.matmul(
                        ps[r + 1][64:128, :],
                        lhsT=(D1ph if ca == 0 else D1h),
                        rhs=view1(ca), start=False, stop=True,
                    )

                stripe = stripes[g]
                # psum already holds y/s (scale folded into weights);
                # both engines cast f32->int8 RNE with saturation.
                if sz == GRP:
                    # two 1024-col pair casts: DVE gets the EARLY pair
                    # (ready two matmuls before group end); ACT, whose
                    # psum reads never disturb the PE, trails at the
                    # group boundary
                    nc.vector.tensor_copy(stripe[:, 0 : 2 * Bs], pr[0])
                    nc.scalar.activation(
                        stripe[:, 2 * Bs : 4 * Bs], pr[1],
                        mybir.ActivationFunctionType.Copy,
                    )
                else:
                    # tail groups: one single cast per engine, in
                    # parallel, right behind the two stop-matmuls
                    nc.vector.tensor_copy(stripe[:, 0:Bs], ps[0])
                    nc.scalar.activation(
                        stripe[:, Bs : 2 * Bs], ps[1],
                        mybir.ActivationFunctionType.Copy,
                    )
                # store issues go to Sync (no cast duties; its loads
                # drain early) so ACT/DVE never delay a psum turnaround;
                # the very last group splits across both rings so the
                # final transfers are 64 KB each and the issues overlap
                if g == len(groups) - 1:
                    nc.sync.dma_start(
                        out=out_d[:, c0 * Bs : (c0 + 1) * Bs],
                        in_=stripe[:, 0:Bs],
                    )
                    nc.scalar.dma_start(
                        out=out_d[:, (c0 + 1) * Bs : (c0 + 2) * Bs],
                        in_=stripe[:, Bs : 2 * Bs],
                    )
                else:
                    nc.sync.dma_start(
                        out=out_d[:, c0 * Bs : (c0 + sz) * Bs],
                        in_=stripe[:, : sz * Bs],
                    )
    nc.compile()
    return nc


def _host_matrices(coefficients: np.ndarray, log_noise_std: np.ndarray):
    """Impulse-response band blocks + initial-state response (f64 host
    math, cast to f32)."""
    n = N_AR
    co = coefficients.astype(np.float64)
    std = float(np.exp(log_noise_std.astype(np.float64))[0])
    L = 256
    h = np.zeros(L, np.float64)
    h[0] = 1.0
    for k in range(1, L):
        for j in range(1, min(k, n) + 1):
            h[k] += co[n - j] * h[k - j]
    hs = h * std
    # band matrix: Hm[k, tau] = h[tau - k] * std;  D0 = Hm[:, :128],
    # D1 = Hm[:, 128:256]
    kk = np.arange(128)[:, None]
    tt = np.arange(256)[None, :]
    d = tt - kk
    m = (d >= 0) & (d < L)
    blk = np.zeros((128, 256), np.float64)
    blk[m] = hs[d[m]]
    Hm = blk.astype(np.float32)
    # G[i, t]: response at time t to unit initial value at slot i
    G = np.zeros((n, 256), np.float64)
    G[:, :n] = np.eye(n)
    for t in range(n, 256):
        G[:, t] = G[:, t - n : t] @ co
    return Hm, np.ascontiguousarray(G.astype(np.float32))


def kernel(initial_values, coefficients, log_noise_std, noise, steps):
    import ml_dtypes

    from concourse.bass_utils import run_bass_kernel_spmd

    global LAST_RESULTS

    initial_values = np.asarray(initial_values, dtype=np.float32)
    coefficients = np.asarray(coefficients, dtype=np.float32)
    log_noise_std = np.asarray(log_noise_std, dtype=np.float32)
    noise = np.asarray(noise, dtype=np.float32)

    Hm, Gm = _host_matrices(coefficients, log_noise_std)
    bf = ml_dtypes.bfloat16

    # pad noise by n rows carrying the INITIAL VALUES (the G-response is
    # spliced into rows 0..15 of chunks 0/1's stationaries), pack
    # time-chunk-major: npk[p, c*Bs + b] = noise2[c*128 + p, b]
    # noise travels as fp8 e3m4 (4-bit mantissa)
    e3 = ml_dtypes.float8_e3m4
    noise2 = np.zeros((STEPS, B_FULL), e3)
    noise2[N_AR:] = noise.astype(e3)
    noise2[:N_AR] = initial_values.T.astype(e3)
    npk_full = np.ascontiguousarray(
        noise2.reshape(NCH, P, B_FULL).transpose(1, 0, 2)
    )  # (128, 64, B_FULL)
    # int8 output scale folded into the weights: psum = y / SCALE
    H = Hm / SCALE
    G = Gm / SCALE
    D0f = H[:, 0:128]
    D1f = H[:, 128:256]
    D0pf = D0f.copy()
    D0pf[:N_AR] = G[:, 0:128]               # G0 rows spliced in
    D1pf = D1f.copy()
    D1pf[:N_AR] = G[:, 128:256]             # G1 rows spliced in
    cmb = np.zeros((P, 512), np.float32)
    cmb[:, 0:128] = D0f
    cmb[:, 128:256] = np.roll(D0f, -64, axis=1)   # D0r (odd chunks)
    cmb[:, 256:320] = D1f[:, 0:64]                # D1h
    cmb[:, 320:448] = D0pf                        # D0p (chunk 0)
    cmb[:, 448:512] = D1pf[:, 0:64]               # D1ph (chunk 1)
    cmb = cmb.astype(bf)

    nc = _build_nc(B_SHARD, NCH)
    in_maps = []
    for c in range(N_CORES):
        sl = slice(B_SHARD * c, B_SHARD * (c + 1))
        in_maps.append(
            {
                "npk": np.ascontiguousarray(npk_full[:, :, sl]).reshape(
                    P, NCH * B_SHARD
                ),
                "cmb": cmb,
            }
        )

    trace = os.environ.get("KERNEL_TRACE", "0") == "1"
    res = run_bass_kernel_spmd(
        nc, in_maps, core_ids=list(range(N_CORES)), trace=trace
    )
    LAST_RESULTS = res

    out = np.empty((B_FULL, STEPS), np.float32)
    for c in range(N_CORES):
        q = np.asarray(res.results[c]["out"]).reshape(P, NCH, B_SHARD)
        # y[b, cc*128 + p] = q[p, cc, b] * SCALE; odd chunks were
        # computed in the rotated psum layout (partition p holds
        # t = (p+64) % 128) and are un-rotated here
        full = q.transpose(1, 0, 2).astype(np.float32) * SCALE
        full[1::2] = np.roll(full[1::2], -64, axis=1)
        out[B_SHARD * c : B_SHARD * (c + 1), :] = full.transpose(
            2, 0, 1
        ).reshape(B_SHARD, STEPS)
    out[:, :N_AR] = initial_values
    return out
